# revision 14
# baseline (speedup 1.0000x reference)
"""MoE routing gate kernel for Trainium2 (8 NeuronCores, data-parallel).

Computes, for x[32768, 2048], weight[64, 2048], bias[64]:
    logits = x @ weight.T
    probs  = softmax(logits, axis=-1)
    idx    = top_k(probs + bias, 6).indices
    w      = take_along_axis(probs, idx)
returning (w float32 [32768, 6], idx int32 [32768, 6]).

Sharding: tokens split 4096/core across 8 cores; weight/bias replicated.

Per-core pipeline (memory-bound; HBM floor ~66us for the 24.6MB shard):
  - x streams at 3 bytes/element (fp16 hi + fp8e4m3 lo,
    lo = (x-fp16(x))*2048); three matmul passes accumulate fp32 logits:
    hi @ w_hi + hi @ w_lo + lo8 @ (w_hi/2048). |logit err| ~2.5e-5 is
    REQUIRED: top-k index flips grow ~linearly in logit error (gaps at
    the rank-6 boundary are ~1e-4) and the graded rel-err on the index
    output is quadratic in flips; 2-byte encodings measurably fail.
  - All weights ride ONE packed tensor issued on the sync ring AHEAD of
    x (FIFO): they land in ~2.5us at full stream rate. On their own
    dynamic queue they'd round-robin against x packets and finish at
    ~25us, gating the early matmul passes and stalling shared
    DMA-semaphore lanes.
  - ~24 dummy matmuls at t~7us (during the dead head before the first
    x chunk lands) trip the PE's HAM clock gate so real matmuls run at
    2.4GHz from the start instead of 1.2GHz.
  - Super-groups: 3x1024 tokens then 2x512. Matmul pairs are
    column-tiled (two half-width matmuls stream concurrently through PE
    column groups 0-63/64-127).
  - The 512-token groups contract over K=124 windows (17 chunks: 16x124
    + 1x64): partitions 124-127 get no x bytes there. DMA engine 15
    (E79) serves partitions {92-95,124-127} AND hosts the dynamic queue
    heads, making it ~15% slower than its peers - it is the stream
    straggler. Halving its share on 25% of the stream rebalances the
    engines (optimum skew fraction ~0.11). The K=64 remainder chunk is
    consumed LAST: only ~96KB and 3 matmul passes remain after the
    final HBM byte, shrinking the serial tail.
  - Per super-group finish (deferred one sg so transposes slot into the
    next sg's DMA-wait bubbles): logits^T -> ACT copy -> PE transposes
    into two PSUM banks (bases 0/64 must not share a bank - hangs HW),
    per-j ACT exp (accum_out = row sum), DVE q = exp + sum*bias (ranks
    identically to probs + bias), Max8/MaxIndex8.
  - Output: one packed [128, nj, 15] f32 tile per sg: cols 0-7 top-8 q,
    8-13 top-6 indices (u32->f32 cast, exact for idx<64), 14 exp-sum.
    ~250KB/core vs 1.1MB for shipping all exp values; the host
    reconstructs w_k = (q_k - sum*bias[idx_k]) / sum.
"""

import numpy as np
import ml_dtypes

import concourse.bacc as bacc
import concourse.bass as bass
import concourse.mybir as mybir
import concourse.tile as tile
from concourse.bass_utils import run_bass_kernel_spmd

F32 = mybir.dt.float32
F16 = mybir.dt.float16
F8E4 = mybir.dt.float8e4
U32 = mybir.dt.uint32
OP = mybir.AluOpType
EXP = mybir.ActivationFunctionType.Exp

TOKENS, DIM, E, TOPK, NCORES = 32768, 2048, 64, 6, 8
KC = DIM // 128          # contraction chunks of 128 (a-group)
SGS_A = 3                # super-groups of 1024 tokens
SGT_A, KQ_A = 1024, 4    # chunk = [128, KQ, sgt]
SGS_B = 2                # trailing super-groups of 512 tokens
SGT_B, KQ_B = 512, 8
KB = 124                 # b-group contraction window (E79 skew)
NB = 16                  # b-group K=124 chunks; remainder K=64 chunk after
PACKW = 15               # mx8 | mi6 | sum
WARMUP_MM = 24           # dummy matmuls at t~7us to trip HAM to 2.4GHz


def build_nc():
    nc = bacc.Bacc("TRN2", target_bir_lowering=False, debug=False)

    xhi_a = nc.dram_tensor(
        "xhi_a", [SGS_A, KC // KQ_A, 128, KQ_A, SGT_A], F16, kind="ExternalInput"
    )
    xlo_a = nc.dram_tensor(
        "xlo_a", [SGS_A, KC // KQ_A, 128, KQ_A, SGT_A], F8E4, kind="ExternalInput"
    )
    xhi_b = nc.dram_tensor(
        "xhi_b", [SGS_B, NB // KQ_B, KB, KQ_B, SGT_B], F16, kind="ExternalInput"
    )
    xlo_b = nc.dram_tensor(
        "xlo_b", [SGS_B, NB // KQ_B, KB, KQ_B, SGT_B], F8E4, kind="ExternalInput"
    )
    xhi_b2 = nc.dram_tensor(
        "xhi_b2", [SGS_B, 64, 1, SGT_B], F16, kind="ExternalInput"
    )
    xlo_b2 = nc.dram_tensor(
        "xlo_b2", [SGS_B, 64, 1, SGT_B], F8E4, kind="ExternalInput"
    )
    w_all = nc.dram_tensor("w_all", [128, 3, KC, E], F16, kind="ExternalInput")
    wb_all = nc.dram_tensor("wb_all", [128, 3, NB + 1, E], F16, kind="ExternalInput")
    misc = nc.dram_tensor("misc", [128, 2, 64], F32, kind="ExternalInput")
    o_pk_a = nc.dram_tensor(
        "o_pk_a", [SGS_A, 128, SGT_A // 128, PACKW], F32, kind="ExternalOutput"
    )
    o_pk_b = nc.dram_tensor(
        "o_pk_b", [SGS_B, 128, SGT_B // 128, PACKW], F32, kind="ExternalOutput"
    )

    with tile.TileContext(nc) as tc:
        with (
            tc.tile_pool(name="consts", bufs=1) as cpool,
            tc.tile_pool(name="xha", bufs=8) as xhap,
            tc.tile_pool(name="xla", bufs=8) as xlap,
            tc.tile_pool(name="xhb", bufs=4) as xhbp,
            tc.tile_pool(name="xlb", bufs=4) as xlbp,
            tc.tile_pool(name="xb2", bufs=2) as xb2p,
            tc.tile_pool(name="lt", bufs=3) as ltp,
            tc.tile_pool(name="ex", bufs=3) as exp_,
            tc.tile_pool(name="wk", bufs=2) as wkp,
            tc.tile_pool(name="small", bufs=3) as smp,
            tc.tile_pool(name="acc", bufs=3, space="PSUM") as accp,
            tc.tile_pool(name="wup", bufs=1, space="PSUM") as wupp,
            tc.tile_pool(name="tr", bufs=2, space="PSUM") as trp,
        ):
            # PE warm-up on memset dummies; no DMA dependencies so the
            # Tensor queue runs these right after the preamble barrier.
            wdum = cpool.tile([128, 64], F16)
            nc.vector.memset(wdum, 0)
            xdum = cpool.tile([128, 512], F16)
            nc.vector.memset(xdum, 0)
            wup = wupp.tile([128, 512], F32, tag="wup")
            for _ in range(WARMUP_MM):
                nc.tensor.matmul(wup[0:64], wdum, xdum, start=True, stop=True)

            # weights FIRST on the sync ring (see module docstring)
            cw = cpool.tile([128, 3, KC, E], F16)
            nc.sync.dma_start(cw, w_all[:])
            cmisc = cpool.tile([128, 2, 64], F32)
            nc.sync.dma_start(cmisc, misc[:])
            cbias = cmisc[:, 0]
            cident = cmisc[:, 1]
            cwb = cpool.tile([128, 3, NB + 1, E], F16)

            def finish_sg(out_dram, idx, acc, sgt):
                """Transpose/softmax/rank/pack for a finished super-group."""
                nj = sgt // 128
                half = nj // 2
                grp = sgt // 2

                lt = ltp.tile([128, 512], F32, tag="lt")
                nc.scalar.copy(lt[0:64, 0:grp], acc[0:64, 0:grp])
                nc.scalar.copy(lt[64:128, 0:grp], acc[64:128, 0:grp])

                # transposes into two PSUM tiles; tiles are padded to a
                # full 2KB bank so base-0 and base-64 reads never share a
                # bank (sharing hangs the HW).
                tpsA = trp.tile([128, 8, E], F32, tag="tpsA")
                tpsB = trp.tile([128, 8, E], F32, tag="tpsB")
                for j in range(nj):
                    base = 64 * (j // half)
                    tps = tpsA if j < half else tpsB
                    nc.tensor.transpose(
                        tps[:, j % half],
                        lt[base:base + 64, (j % half) * 128:(j % half + 1) * 128],
                        cident[base:base + 64, :],
                    )

                ex = exp_.tile([128, 8, E], F32, tag="ex")
                q = wkp.tile([128, 8, E], F32, tag="q")
                pk = smp.tile([128, 8, 16], F32, tag="pk")
                mi = smp.tile([128, 8, 8], U32, tag="mi")
                for j in range(nj):
                    tps = (tpsA if j < half else tpsB)[:, j % half]
                    nc.scalar.activation(
                        ex[:, j], tps, EXP, accum_out=pk[:, j, 14:15]
                    )
                    nc.vector.scalar_tensor_tensor(
                        q[:, j], cbias, pk[:, j, 14:15], ex[:, j],
                        OP.mult, OP.add,
                    )
                    nc.vector.max(pk[:, j, 0:8], q[:, j])
                    nc.vector.max_index(mi[:, j], pk[:, j, 0:8], q[:, j])
                    # u32 -> f32 value cast; exact for idx < 64
                    nc.vector.tensor_copy(pk[:, j, 8:14], mi[:, j, 0:TOPK])

                nc.gpsimd.dma_start(out_dram[idx], pk[:, 0:nj, 0:PACKW])

            pending = None  # (out_dram, idx, acc, sgt) awaiting finish

            def run_sg(out_dram, idx, sgt, chunks, wtile, ksizes):
                """chunks: list of (hi_tile, lo_tile, nk); ksizes: K per
                global chunk index."""
                nonlocal pending
                grp = sgt // 2
                acc = accp.tile([128, 512], F32)
                nchunks = len(ksizes)
                kk = 0
                first_nk = chunks[0][2]
                for th, tl, nk in chunks:
                    for s in range(nk):
                        K = ksizes[kk]
                        hi_k = th[0:K, s]   # [K, sgt] fp16
                        lo_k = tl[0:K, s]   # [K, sgt] fp8
                        for p in range(3):
                            w = wtile[0:K, p, kk, :]
                            xs = (hi_k, hi_k, lo_k)[p]
                            first = kk == 0 and p == 0
                            last = kk == nchunks - 1 and p == 2
                            nc.tensor.matmul(
                                acc[0:64, 0:grp], w, xs[:, 0:grp],
                                start=first, stop=last, tile_position=(0, 0),
                            )
                            nc.tensor.matmul(
                                acc[64:128, 0:grp], w, xs[:, grp:sgt],
                                start=first, stop=last, tile_position=(0, 64),
                                skip_group_check=True,
                            )
                        kk += 1
                        if kk == first_nk and pending is not None:
                            # issue previous sg's finish after this sg's
                            # first chunk: the in-order Tensor queue then
                            # runs its transposes inside DMA-wait bubbles
                            finish_sg(*pending)
                            pending = None
                pending = (out_dram, idx, acc, sgt)

            for i in range(SGS_A):
                chunks = []
                for c in range(KC // KQ_A):
                    th = xhap.tile([128, KQ_A, SGT_A], F16, tag="xh")
                    nc.sync.dma_start(th, xhi_a[i, c])
                    tl = xlap.tile([128, KQ_A, SGT_A], F8E4, tag="xl")
                    nc.sync.dma_start(tl, xlo_a[i, c])
                    chunks.append((th, tl, KQ_A))
                    if i == 0 and c == 0:
                        # b-group weights ride in-stream, needed only late
                        nc.sync.dma_start(cwb, wb_all[:])
                run_sg(o_pk_a, i, SGT_A, chunks, cw, [128] * KC)

            for i in range(SGS_B):
                chunks = []
                for c in range(NB // KQ_B):
                    th = xhbp.tile([KB, KQ_B, SGT_B], F16, tag="xh")
                    nc.sync.dma_start(th, xhi_b[i, c])
                    tl = xlbp.tile([KB, KQ_B, SGT_B], F8E4, tag="xl")
                    nc.sync.dma_start(tl, xlo_b[i, c])
                    chunks.append((th, tl, KQ_B))
                th2 = xb2p.tile([64, 1, SGT_B], F16, tag="xh2")
                nc.sync.dma_start(th2, xhi_b2[i])
                tl2 = xb2p.tile([64, 1, SGT_B], F8E4, tag="xl2")
                nc.sync.dma_start(tl2, xlo_b2[i])
                chunks.append((th2, tl2, 1))
                run_sg(
                    o_pk_b, i, SGT_B, chunks, cwb, [KB] * NB + [64]
                )

            finish_sg(*pending)
    return nc


_CACHE = {}


def _get_compiled():
    if "nc" not in _CACHE:
        nc = build_nc()
        nc.compile()
        _CACHE["nc"] = nc
    return _CACHE["nc"]


def _prep_shared(weight, bias):
    w = np.asarray(weight, np.float32)
    w_hi = w.astype(np.float16)
    w_lo = (w - w_hi.astype(np.float32)).astype(np.float16)
    w_3 = (w_hi.astype(np.float32) * (1.0 / 2048.0)).astype(np.float16)

    def wtile(a):  # [E, DIM] -> [128, KC, E]
        return np.ascontiguousarray(a.T).reshape(KC, 128, E).transpose(1, 0, 2)

    def wbtile(a):  # [E, DIM] -> [128, NB+1, E] with K=124 chunking
        aT = np.ascontiguousarray(a.T)  # [DIM, E]
        out = np.zeros((128, NB + 1, E), a.dtype)
        for c in range(NB):
            out[0:KB, c] = aT[c * KB:(c + 1) * KB]
        out[0:64, NB] = aT[NB * KB:DIM]
        return out

    w_all = np.ascontiguousarray(
        np.stack([wtile(v) for v in (w_hi, w_lo, w_3)], axis=1)
    )
    wb_all = np.ascontiguousarray(
        np.stack([wbtile(v) for v in (w_hi, w_lo, w_3)], axis=1)
    )
    misc = np.empty((128, 2, 64), np.float32)
    misc[:, 0] = np.asarray(bias, np.float32)
    misc[:, 1] = np.tile(np.eye(64, dtype=np.float32), (2, 1))
    return {"w_all": w_all, "wb_all": wb_all, "misc": np.ascontiguousarray(misc)}


def prep_core_inputs(x, weight, bias, ncores=NCORES):
    shared = _prep_shared(weight, bias)
    x = np.asarray(x, np.float32)
    tpc = x.shape[0] // ncores
    na = SGS_A * SGT_A
    # whole-tensor transpose + casts once (not per core)
    xT = np.ascontiguousarray(x.T)           # [DIM, TOKENS]
    xhT = xT.astype(np.float16)
    xlT = ((xT - xhT.astype(np.float32)) * 2048.0).astype(
        ml_dtypes.float8_e4m3fn
    )
    del xT

    def pack_a(xx):  # [DIM, SGS_A*1024] -> [SGS_A, KC//KQ, 128, KQ, SGT]
        x6 = xx.reshape(KC // KQ_A, KQ_A, 128, SGS_A, SGT_A)
        return np.ascontiguousarray(x6.transpose(3, 0, 2, 1, 4))

    def pack_b(xx):  # [DIM, SGS_B*512] main part, K=124 chunks
        x6 = xx[0:NB * KB].reshape(NB // KQ_B, KQ_B, KB, SGS_B, SGT_B)
        return np.ascontiguousarray(x6.transpose(3, 0, 2, 1, 4))

    def pack_b2(xx):  # remainder dims 1984:2048
        return np.ascontiguousarray(
            xx[NB * KB:DIM].reshape(64, SGS_B, SGT_B).transpose(1, 0, 2)
        )[:, :, None, :]

    in_maps = []
    for c in range(ncores):
        lo = c * tpc
        ah, al = xhT[:, lo:lo + na], xlT[:, lo:lo + na]
        bh, bl = xhT[:, lo + na:lo + tpc], xlT[:, lo + na:lo + tpc]
        in_maps.append({
            "xhi_a": pack_a(ah), "xlo_a": pack_a(al),
            "xhi_b": pack_b(bh), "xlo_b": pack_b(bl),
            "xhi_b2": pack_b2(bh), "xlo_b2": pack_b2(bl),
            **shared,
        })
    return in_maps


def unpack_outputs(res_list, bias):
    bias = np.asarray(bias, np.float64)
    ws, idxs = [], []
    for r in res_list:
        for nm in ("o_pk_a", "o_pk_b"):
            pk = np.asarray(r[nm], np.float64)  # [nsg, 128, nj, 15]
            # token t = sg*sgt + 128*j + p
            pk = pk.transpose(0, 2, 1, 3).reshape(-1, PACKW)
            mx = pk[:, 0:TOPK]
            mi = np.rint(pk[:, 8:14]).astype(np.int64)
            ssum = pk[:, 14:15]
            wv = (mx - ssum * bias[mi]) / ssum
            ws.append(wv)
            idxs.append(mi)
    return (
        np.ascontiguousarray(np.concatenate(ws)).astype(np.float32),
        np.ascontiguousarray(np.concatenate(idxs)).astype(np.int32),
    )


def run(x, weight, bias, trace=False, **kwargs):
    x = np.asarray(x, np.float32)
    nc = _get_compiled()
    in_maps = prep_core_inputs(x, weight, bias)
    res = run_bass_kernel_spmd(
        nc, in_maps, list(range(NCORES)), trace=trace, **kwargs
    )
    w, i = unpack_outputs(res.results, bias)
    return w, i, res


def kernel(x, weight, bias):
    w, i, _ = run(x, weight, bias, trace=False)
    return w, i


# revision 22
# speedup vs baseline: 1.4070x; 1.4070x over previous
"""MoE routing gate kernel for Trainium2 (8 NeuronCores, data-parallel).

Computes, for x[32768, 2048], weight[64, 2048], bias[64]:
    logits = x @ weight.T
    probs  = softmax(logits, axis=-1)
    idx    = top_k(probs + bias, 6).indices
    w      = take_along_axis(probs, idx)
returning (w float32 [32768, 6], idx int32 [32768, 6]).

Sharding: tokens split 4096/core across 8 cores; weight/bias replicated.

Per-core pipeline (memory-bound; HBM floor ~66us for the 24.6MB shard):
  - x streams at 3 bytes/element (fp16 hi + fp8e4m3 lo,
    lo = (x-fp16(x))*2048); three matmul passes accumulate fp32 logits:
    hi @ w_hi + hi @ w_lo + lo8 @ (w_hi/2048). |logit err| ~2.5e-5 is
    REQUIRED: top-k index flips grow ~linearly in logit error (gaps at
    the rank-6 boundary are ~1e-4) and the graded rel-err on the index
    output is quadratic in flips; 2-byte encodings measurably fail.
  - All weights ride ONE packed tensor issued on the sync ring AHEAD of
    x (FIFO): they land in ~2.5us at full stream rate. On their own
    dynamic queue they'd round-robin against x packets and finish at
    ~25us, gating the early matmul passes and stalling shared
    DMA-semaphore lanes.
  - ~24 dummy matmuls at t~7us (during the dead head before the first
    x chunk lands) trip the PE's HAM clock gate so real matmuls run at
    2.4GHz from the start instead of 1.2GHz.
  - Super-groups: 3x1024 tokens then 2x512. Matmul pairs are
    column-tiled (two half-width matmuls stream concurrently through PE
    column groups 0-63/64-127).
  - HWDGE DMA fan-out rule (probed): a [P, ...] descriptor splits into
    n = largest-divisor-of-P <= 16 consecutive partition slices,
    assigned to engines E64..E63+n in order. DMA engine 15 (E79) hosts
    the dynamic queue heads and runs ~15% slower than its peers - it is
    the stream straggler. The LAST 512-token group contracts over
    K=120 windows (120 = 15 slices of 8 -> E79 gets ZERO bytes), which
    rebalances per-engine finish times (optimum skew ~= one 512-group).
    Its final chunk is a normal K=128 one, consumed LAST: only ~200KB
    and 3 matmul passes remain after the final HBM byte, shrinking the
    serial tail.
  - Per super-group finish (deferred one sg so transposes slot into the
    next sg's DMA-wait bubbles): logits^T -> ACT copy -> PE transposes
    into two PSUM banks (bases 0/64 must not share a bank - hangs HW),
    per-j ACT exp (accum_out = row sum), DVE q = exp + sum*bias (ranks
    identically to probs + bias), Max8/MaxIndex8.
  - Output: one packed [128, nj, 15] f32 tile per sg: cols 0-7 top-8 q,
    8-13 top-6 indices (u32->f32 cast, exact for idx<64), 14 exp-sum.
    ~250KB/core vs 1.1MB for shipping all exp values; the host
    reconstructs w_k = (q_k - sum*bias[idx_k]) / sum.
"""

import numpy as np
import ml_dtypes

import concourse.bacc as bacc
import concourse.bass as bass
import concourse.mybir as mybir
import concourse.tile as tile
from concourse.bass_utils import run_bass_kernel_spmd

F32 = mybir.dt.float32
F16 = mybir.dt.float16
F8E4 = mybir.dt.float8e4
U32 = mybir.dt.uint32
OP = mybir.AluOpType
EXP = mybir.ActivationFunctionType.Exp

TOKENS, DIM, E, TOPK, NCORES = 32768, 2048, 64, 6, 8
KC = DIM // 128          # contraction chunks of 128 (a-group)
SGS_A = 3                # super-groups of 1024 tokens
SGT_A, KQ_A = 1024, 4    # chunk = [128, KQ, sgt]
SGS_B = 2                # trailing super-groups of 512 tokens
SGT_B, KQ_B = 512, 8
KB = 120                 # b1 contraction window: 120 = 15 engines x 8
                         # partition-lines -> DMA engine 15 (E79, the
                         # straggler) gets ZERO bytes for these chunks
NB = 16                  # b1 K=120 chunks; then one full K=128 chunk
PACKW = 15               # mx8 | mi6 | sum
WARMUP_MM = 24           # dummy matmuls at t~7us to trip HAM to 2.4GHz


def build_nc():
    nc = bacc.Bacc("TRN2", target_bir_lowering=False, debug=False)

    xhi_a = nc.dram_tensor(
        "xhi_a", [SGS_A, KC // KQ_A, 128, KQ_A, SGT_A], F16, kind="ExternalInput"
    )
    xlo_a = nc.dram_tensor(
        "xlo_a", [SGS_A, KC // KQ_A, 128, KQ_A, SGT_A], F8E4, kind="ExternalInput"
    )
    xhi_b0 = nc.dram_tensor(
        "xhi_b0", [NB // KQ_B, 128, KQ_B, SGT_B], F16, kind="ExternalInput"
    )
    xlo_b0 = nc.dram_tensor(
        "xlo_b0", [NB // KQ_B, 128, KQ_B, SGT_B], F8E4, kind="ExternalInput"
    )
    xhi_b1 = nc.dram_tensor(
        "xhi_b1", [NB // KQ_B, KB, KQ_B, SGT_B], F16, kind="ExternalInput"
    )
    xlo_b1 = nc.dram_tensor(
        "xlo_b1", [NB // KQ_B, KB, KQ_B, SGT_B], F8E4, kind="ExternalInput"
    )
    xhi_b1f = nc.dram_tensor("xhi_b1f", [128, 1, SGT_B], F16, kind="ExternalInput")
    xlo_b1f = nc.dram_tensor("xlo_b1f", [128, 1, SGT_B], F8E4, kind="ExternalInput")
    w_all = nc.dram_tensor("w_all", [128, 3, KC, E], F16, kind="ExternalInput")
    wb_all = nc.dram_tensor("wb_all", [128, 3, NB + 1, E], F16, kind="ExternalInput")
    misc = nc.dram_tensor("misc", [128, 2, 64], F32, kind="ExternalInput")
    o_pk_a = nc.dram_tensor(
        "o_pk_a", [SGS_A, 128, SGT_A // 128, PACKW], F32, kind="ExternalOutput"
    )
    o_pk_b = nc.dram_tensor(
        "o_pk_b", [SGS_B, 128, SGT_B // 128, PACKW], F32, kind="ExternalOutput"
    )

    with tile.TileContext(nc) as tc:
        with (
            tc.tile_pool(name="consts", bufs=1) as cpool,
            tc.tile_pool(name="xha", bufs=8) as xhap,
            tc.tile_pool(name="xla", bufs=8) as xlap,
            tc.tile_pool(name="xb0", bufs=2) as xb0p,
            tc.tile_pool(name="xb1", bufs=2) as xb1p,
            tc.tile_pool(name="xb1f", bufs=1) as xb1fp,
            tc.tile_pool(name="lt", bufs=3) as ltp,
            tc.tile_pool(name="ex", bufs=3) as exp_,
            tc.tile_pool(name="wk", bufs=2) as wkp,
            tc.tile_pool(name="small", bufs=3) as smp,
            tc.tile_pool(name="acc", bufs=3, space="PSUM") as accp,
            tc.tile_pool(name="wup", bufs=1, space="PSUM") as wupp,
            tc.tile_pool(name="tr", bufs=2, space="PSUM") as trp,
        ):
            # PE warm-up on memset dummies; no DMA dependencies so the
            # Tensor queue runs these right after the preamble barrier.
            wdum = cpool.tile([128, 64], F16)
            nc.vector.memset(wdum, 0)
            xdum = cpool.tile([128, 512], F16)
            nc.vector.memset(xdum, 0)
            wup = wupp.tile([128, 512], F32, tag="wup")
            for _ in range(WARMUP_MM):
                nc.tensor.matmul(wup[0:64], wdum, xdum, start=True, stop=True)

            # weights FIRST on the sync ring (see module docstring)
            cw = cpool.tile([128, 3, KC, E], F16)
            nc.sync.dma_start(cw, w_all[:])
            cmisc = cpool.tile([128, 2, 64], F32)
            nc.sync.dma_start(cmisc, misc[:])
            cbias = cmisc[:, 0]
            cident = cmisc[:, 1]
            cwb = cpool.tile([128, 3, NB + 1, E], F16)

            def finish_sg(out_dram, idx, acc, sgt):
                """Transpose/softmax/rank/pack for a finished super-group."""
                nj = sgt // 128
                half = nj // 2
                grp = sgt // 2

                lt = ltp.tile([128, 512], F32, tag="lt")
                nc.scalar.copy(lt[0:64, 0:grp], acc[0:64, 0:grp])
                nc.scalar.copy(lt[64:128, 0:grp], acc[64:128, 0:grp])

                # transposes into two PSUM tiles; tiles are padded to a
                # full 2KB bank so base-0 and base-64 reads never share a
                # bank (sharing hangs the HW).
                tpsA = trp.tile([128, 8, E], F32, tag="tpsA")
                tpsB = trp.tile([128, 8, E], F32, tag="tpsB")
                for j in range(nj):
                    base = 64 * (j // half)
                    tps = tpsA if j < half else tpsB
                    nc.tensor.transpose(
                        tps[:, j % half],
                        lt[base:base + 64, (j % half) * 128:(j % half + 1) * 128],
                        cident[base:base + 64, :],
                    )

                ex = exp_.tile([128, 8, E], F32, tag="ex")
                q = wkp.tile([128, 8, E], F32, tag="q")
                pk = smp.tile([128, 8, 16], F32, tag="pk")
                mi = smp.tile([128, 8, 8], U32, tag="mi")
                for j in range(nj):
                    tps = (tpsA if j < half else tpsB)[:, j % half]
                    nc.scalar.activation(
                        ex[:, j], tps, EXP, accum_out=pk[:, j, 14:15]
                    )
                    nc.vector.scalar_tensor_tensor(
                        q[:, j], cbias, pk[:, j, 14:15], ex[:, j],
                        OP.mult, OP.add,
                    )
                    nc.vector.max(pk[:, j, 0:8], q[:, j])
                    nc.vector.max_index(mi[:, j], pk[:, j, 0:8], q[:, j])
                    # u32 -> f32 value cast; exact for idx < 64
                    nc.vector.tensor_copy(pk[:, j, 8:14], mi[:, j, 0:TOPK])

                nc.gpsimd.dma_start(out_dram[idx], pk[:, 0:nj, 0:PACKW])

            pending = None  # (out_dram, idx, acc, sgt) awaiting finish

            def run_sg(out_dram, idx, sgt, chunks, wtile, ksizes):
                """chunks: list of (hi_tile, lo_tile, nk); ksizes: K per
                global chunk index."""
                nonlocal pending
                grp = sgt // 2
                acc = accp.tile([128, 512], F32)
                nchunks = len(ksizes)
                kk = 0
                first_nk = chunks[0][2]
                for th, tl, nk in chunks:
                    for s in range(nk):
                        K = ksizes[kk]
                        hi_k = th[0:K, s]   # [K, sgt] fp16
                        lo_k = tl[0:K, s]   # [K, sgt] fp8
                        for p in range(3):
                            w = wtile[0:K, p, kk, :]
                            xs = (hi_k, hi_k, lo_k)[p]
                            first = kk == 0 and p == 0
                            last = kk == nchunks - 1 and p == 2
                            nc.tensor.matmul(
                                acc[0:64, 0:grp], w, xs[:, 0:grp],
                                start=first, stop=last, tile_position=(0, 0),
                            )
                            nc.tensor.matmul(
                                acc[64:128, 0:grp], w, xs[:, grp:sgt],
                                start=first, stop=last, tile_position=(0, 64),
                                skip_group_check=True,
                            )
                        kk += 1
                        if kk == first_nk and pending is not None:
                            # issue previous sg's finish after this sg's
                            # first chunk: the in-order Tensor queue then
                            # runs its transposes inside DMA-wait bubbles
                            finish_sg(*pending)
                            pending = None
                pending = (out_dram, idx, acc, sgt)

            for i in range(SGS_A):
                chunks = []
                for c in range(KC // KQ_A):
                    th = xhap.tile([128, KQ_A, SGT_A], F16, tag="xh")
                    nc.sync.dma_start(th, xhi_a[i, c])
                    tl = xlap.tile([128, KQ_A, SGT_A], F8E4, tag="xl")
                    nc.sync.dma_start(tl, xlo_a[i, c])
                    chunks.append((th, tl, KQ_A))
                    if i == 0 and c == 0:
                        # b-group weights ride in-stream, needed only late
                        nc.sync.dma_start(cwb, wb_all[:])
                run_sg(o_pk_a, i, SGT_A, chunks, cw, [128] * KC)

            # b0: normal K=128 chunking, reuses the a-group weight tiling
            chunks = []
            for c in range(NB // KQ_B):
                th = xb0p.tile([128, KQ_B, SGT_B], F16, tag="xh")
                nc.sync.dma_start(th, xhi_b0[c])
                tl = xb0p.tile([128, KQ_B, SGT_B], F8E4, tag="xl")
                nc.sync.dma_start(tl, xlo_b0[c])
                chunks.append((th, tl, KQ_B))
            run_sg(o_pk_b, 0, SGT_B, chunks, cw, [128] * KC)

            # b1 (last): K=120 chunks (E79-skewed) + one K=128 tail chunk;
            # the tail chunk is tiny so only ~3 matmul passes + the finish
            # chain remain after the final HBM byte.
            chunks = []
            for c in range(NB // KQ_B):
                th = xb1p.tile([KB, KQ_B, SGT_B], F16, tag="xh")
                nc.sync.dma_start(th, xhi_b1[c])
                tl = xb1p.tile([KB, KQ_B, SGT_B], F8E4, tag="xl")
                nc.sync.dma_start(tl, xlo_b1[c])
                chunks.append((th, tl, KQ_B))
            thf = xb1fp.tile([128, 1, SGT_B], F16, tag="xhf")
            nc.sync.dma_start(thf, xhi_b1f[:])
            tlf = xb1fp.tile([128, 1, SGT_B], F8E4, tag="xlf")
            nc.sync.dma_start(tlf, xlo_b1f[:])
            chunks.append((thf, tlf, 1))
            run_sg(o_pk_b, 1, SGT_B, chunks, cwb, [KB] * NB + [128])

            finish_sg(*pending)
    return nc


_CACHE = {}


def _get_compiled():
    if "nc" not in _CACHE:
        nc = build_nc()
        nc.compile()
        _CACHE["nc"] = nc
    return _CACHE["nc"]


def _prep_shared(weight, bias):
    w = np.asarray(weight, np.float32)
    w_hi = w.astype(np.float16)
    w_lo = (w - w_hi.astype(np.float32)).astype(np.float16)
    w_3 = (w_hi.astype(np.float32) * (1.0 / 2048.0)).astype(np.float16)

    def wtile(a):  # [E, DIM] -> [128, KC, E]
        return np.ascontiguousarray(a.T).reshape(KC, 128, E).transpose(1, 0, 2)

    def wbtile(a):  # [E, DIM] -> [128, NB+1, E] with K=120 chunking
        aT = np.ascontiguousarray(a.T)  # [DIM, E]
        out = np.zeros((128, NB + 1, E), a.dtype)
        for c in range(NB):
            out[0:KB, c] = aT[c * KB:(c + 1) * KB]
        out[:, NB] = aT[NB * KB:DIM]
        return out

    w_all = np.ascontiguousarray(
        np.stack([wtile(v) for v in (w_hi, w_lo, w_3)], axis=1)
    )
    wb_all = np.ascontiguousarray(
        np.stack([wbtile(v) for v in (w_hi, w_lo, w_3)], axis=1)
    )
    misc = np.empty((128, 2, 64), np.float32)
    misc[:, 0] = np.asarray(bias, np.float32)
    misc[:, 1] = np.tile(np.eye(64, dtype=np.float32), (2, 1))
    return {"w_all": w_all, "wb_all": wb_all, "misc": np.ascontiguousarray(misc)}


def prep_core_inputs(x, weight, bias, ncores=NCORES):
    shared = _prep_shared(weight, bias)
    x = np.asarray(x, np.float32)
    tpc = x.shape[0] // ncores
    na = SGS_A * SGT_A
    # whole-tensor transpose + casts once (not per core)
    xT = np.ascontiguousarray(x.T)           # [DIM, TOKENS]
    xhT = xT.astype(np.float16)
    xlT = ((xT - xhT.astype(np.float32)) * 2048.0).astype(
        ml_dtypes.float8_e4m3fn
    )
    del xT

    def pack_a(xx):  # [DIM, SGS_A*1024] -> [SGS_A, KC//KQ, 128, KQ, SGT]
        x6 = xx.reshape(KC // KQ_A, KQ_A, 128, SGS_A, SGT_A)
        return np.ascontiguousarray(x6.transpose(3, 0, 2, 1, 4))

    def pack_b0(xx):  # [DIM, 512] -> [2, 128, 8, 512], K=128 chunks
        x4 = xx.reshape(NB // KQ_B, KQ_B, 128, SGT_B)
        return np.ascontiguousarray(x4.transpose(0, 2, 1, 3))

    def pack_b1(xx):  # [DIM, 512] main -> [2, 120, 8, 512], K=120 chunks
        x4 = xx[0:NB * KB].reshape(NB // KQ_B, KQ_B, KB, SGT_B)
        return np.ascontiguousarray(x4.transpose(0, 2, 1, 3))

    def pack_b1f(xx):  # remainder dims 1920:2048 -> [128, 1, 512]
        return np.ascontiguousarray(xx[NB * KB:DIM])[:, None, :]

    in_maps = []
    for c in range(ncores):
        lo = c * tpc
        ah, al = xhT[:, lo:lo + na], xlT[:, lo:lo + na]
        b0h = xhT[:, lo + na:lo + na + SGT_B]
        b0l = xlT[:, lo + na:lo + na + SGT_B]
        b1h = xhT[:, lo + na + SGT_B:lo + tpc]
        b1l = xlT[:, lo + na + SGT_B:lo + tpc]
        in_maps.append({
            "xhi_a": pack_a(ah), "xlo_a": pack_a(al),
            "xhi_b0": pack_b0(b0h), "xlo_b0": pack_b0(b0l),
            "xhi_b1": pack_b1(b1h), "xlo_b1": pack_b1(b1l),
            "xhi_b1f": pack_b1f(b1h), "xlo_b1f": pack_b1f(b1l),
            **shared,
        })
    return in_maps


def unpack_outputs(res_list, bias):
    bias = np.asarray(bias, np.float64)
    ws, idxs = [], []
    for r in res_list:
        for nm in ("o_pk_a", "o_pk_b"):
            pk = np.asarray(r[nm], np.float64)  # [nsg, 128, nj, 15]
            # token t = sg*sgt + 128*j + p
            pk = pk.transpose(0, 2, 1, 3).reshape(-1, PACKW)
            mx = pk[:, 0:TOPK]
            mi = np.rint(pk[:, 8:14]).astype(np.int64)
            ssum = pk[:, 14:15]
            wv = (mx - ssum * bias[mi]) / ssum
            ws.append(wv)
            idxs.append(mi)
    return (
        np.ascontiguousarray(np.concatenate(ws)).astype(np.float32),
        np.ascontiguousarray(np.concatenate(idxs)).astype(np.int32),
    )


def run(x, weight, bias, trace=False, **kwargs):
    x = np.asarray(x, np.float32)
    nc = _get_compiled()
    in_maps = prep_core_inputs(x, weight, bias)
    res = run_bass_kernel_spmd(
        nc, in_maps, list(range(NCORES)), trace=trace, **kwargs
    )
    w, i = unpack_outputs(res.results, bias)
    return w, i, res


def kernel(x, weight, bias):
    w, i, _ = run(x, weight, bias, trace=False)
    return w, i
